# revision 20
# baseline (speedup 1.0000x reference)
import sys

sys.path.insert(0, "/opt/trn_rl_repo")

import numpy as np
import ml_dtypes

import concourse.bass as bass
import concourse.bacc as bacc
import concourse.tile as tile
from concourse.bass_utils import run_bass_kernel_spmd
from concourse import mybir

B, L, D, H = 2, 2048, 1024, 16
DH = 64          # dim per head
HPC = 4          # heads per core
CPC = HPC * DH   # feature cols per core = 256
NCORES = 8

MM_DT = "bfloat16"
NP_MM = ml_dtypes.bfloat16 if MM_DT == "bfloat16" else np.float32

_CACHE = {}


def build_nc(mm_dt: str):
    nc = bacc.Bacc()
    mm_dt = mybir.dt(mm_dt)
    fp32 = mybir.dt.float32

    # External I/O (per core views, already sharded on host)
    xq = nc.declare_dram_parameter("xq", (D, L), mm_dt, isOutput=False)   # q[b].T
    xk = nc.declare_dram_parameter("xk", (D, L), mm_dt, isOutput=False)   # k[b].T
    xv = nc.declare_dram_parameter("xv", (D, L), mm_dt, isOutput=False)   # v[b].T
    wq = nc.declare_dram_parameter("wq", (D, CPC), mm_dt, isOutput=False)
    wk = nc.declare_dram_parameter("wk", (D, CPC), mm_dt, isOutput=False)
    wv = nc.declare_dram_parameter("wv", (D, CPC), mm_dt, isOutput=False)
    wo = nc.declare_dram_parameter("wo", (CPC, D), mm_dt, isOutput=False)
    bq = nc.declare_dram_parameter("bq", (CPC, 1), fp32, isOutput=False)
    bk = nc.declare_dram_parameter("bk", (CPC, 1), fp32, isOutput=False)
    y = nc.declare_dram_parameter("y", (L, D), fp32, isOutput=True)       # partial out

    from contextlib import ExitStack

    with ExitStack() as es:
        tc = es.enter_context(tile.TileContext(nc))
        # NOTE: bufs are per named tag
        xt_pool = es.enter_context(tc.tile_pool(name="xt", bufs=1))     # 3 tags [128,8,2048]
        w_pool = es.enter_context(tc.tile_pool(name="w", bufs=1))       # 3 tags [128,8,256]
        wo_pool = es.enter_context(tc.tile_pool(name="wo", bufs=1))     # 2 tags [128,1024]
        bias_pool = es.enter_context(tc.tile_pool(name="bias", bufs=1))
        qt_pool = es.enter_context(tc.tile_pool(name="qt", bufs=1))     # 2 tags [128,2048]
        kt_pool = es.enter_context(tc.tile_pool(name="kt", bufs=1))
        vn_pool = es.enter_context(tc.tile_pool(name="vn", bufs=1))     # [128,16,4,65]
        ones_pool = es.enter_context(tc.tile_pool(name="ones", bufs=1))
        pt_pool = es.enter_context(tc.tile_pool(name="pt", bufs=3))     # [128,512]
        zr_pool = es.enter_context(tc.tile_pool(name="zr", bufs=2))     # [1,512]
        zbs_pool = es.enter_context(tc.tile_pool(name="zbs", bufs=2))   # [64,512]
        ot_pool = es.enter_context(tc.tile_pool(name="ot", bufs=1))     # 2 tags [128,2048]
        y_pool = es.enter_context(tc.tile_pool(name="ysb", bufs=2))     # [128,4,1024]
        psA = es.enter_context(tc.tile_pool(name="psA", bufs=2, space="PSUM"))
        psS = es.enter_context(tc.tile_pool(name="psS", bufs=2, space="PSUM"))
        psOT = es.enter_context(tc.tile_pool(name="psOT", bufs=2, space="PSUM"))
        psZ = es.enter_context(tc.tile_pool(name="psZ", bufs=2, space="PSUM"))
        if True:
            # ---- load inputs -------------------------------------------------
            xq_sb = xt_pool.tile([128, 8, L], mm_dt)
            xk_sb = xt_pool.tile([128, 8, L], mm_dt)
            xv_sb = xt_pool.tile([128, 8, L], mm_dt)
            nc.sync.dma_start(out=xq_sb, in_=xq.rearrange("(dc p) c -> p dc c", p=128))
            nc.sync.dma_start(out=xk_sb, in_=xk.rearrange("(dc p) c -> p dc c", p=128))
            nc.sync.dma_start(out=xv_sb, in_=xv.rearrange("(dc p) c -> p dc c", p=128))

            wq_sb = w_pool.tile([128, 8, CPC], mm_dt)
            wk_sb = w_pool.tile([128, 8, CPC], mm_dt)
            wv_sb = w_pool.tile([128, 8, CPC], mm_dt)
            nc.sync.dma_start(out=wq_sb, in_=wq.rearrange("(dc p) c -> p dc c", p=128))
            nc.sync.dma_start(out=wk_sb, in_=wk.rearrange("(dc p) c -> p dc c", p=128))
            nc.sync.dma_start(out=wv_sb, in_=wv.rearrange("(dc p) c -> p dc c", p=128))

            wo_sb = []
            for cc in range(2):
                t = wo_pool.tile([128, D], mm_dt, name=f"wo{cc}")
                nc.sync.dma_start(out=t, in_=wo[cc * 128:(cc + 1) * 128, :])
                wo_sb.append(t)

            bq_sb = bias_pool.tile([128, 2], fp32)
            bk_sb = bias_pool.tile([128, 2], fp32)
            nc.sync.dma_start(out=bq_sb, in_=bq.rearrange("(cc p) o -> p cc o", p=128))
            nc.sync.dma_start(out=bk_sb, in_=bk.rearrange("(cc p) o -> p cc o", p=128))

            ones_sb = ones_pool.tile([1, 64], fp32)
            nc.vector.memset(ones_sb, 1.0)

            # ---- stage A: projections ---------------------------------------
            # Q^T, K^T: [CPC(part), L(free)] as 2 chunk tiles [128, 2048]
            qt_sb = [qt_pool.tile([128, L], mm_dt, name=f"qt{i}") for i in range(2)]
            kt_sb = [kt_pool.tile([128, L], mm_dt, name=f"kt{i}") for i in range(2)]
            for dst, x_sb, w_sb, b_sb in (
                (qt_sb, xq_sb, wq_sb, bq_sb),
                (kt_sb, xk_sb, wk_sb, bk_sb),
            ):
                for cc in range(2):
                    for lg in range(4):
                        ps = psA.tile([128, 512], fp32)
                        for dc in range(8):
                            nc.tensor.matmul(
                                ps,
                                w_sb[:, dc, cc * 128:(cc + 1) * 128],
                                x_sb[:, dc, lg * 512:(lg + 1) * 512],
                                start=(dc == 0),
                                stop=(dc == 7),
                            )
                        nc.vector.tensor_scalar_add(
                            out=dst[cc][:, lg * 512:(lg + 1) * 512],
                            in0=ps,
                            scalar1=b_sb[:, cc:cc + 1],
                        )

            # V natural layout: [128(lt-part), 16 lt, 4 head, 65] (col 64 = ones)
            v_sb = vn_pool.tile([128, 16, 4, 65], mm_dt)
            nc.vector.memset(v_sb[:, :, :, 64:65], 1.0)
            for lt in range(16):
                ps = psA.tile([128, CPC], fp32)
                for dc in range(8):
                    nc.tensor.matmul(
                        ps,
                        xv_sb[:, dc, lt * 128:(lt + 1) * 128],
                        wv_sb[:, dc, :],
                        start=(dc == 0),
                        stop=(dc == 7),
                    )
                nc.vector.tensor_copy(
                    out=v_sb[:, lt, :, 0:64],
                    in_=ps.rearrange("p (h d) -> p h d", d=64),
                )

            # ---- stage B: attention (S^T layout, causal) --------------------
            # out^T accumulates per (head, 512-q-group); row 64 = Z (denominator)
            ot_sb = [ot_pool.tile([128, L], mm_dt, name=f"ot{i}") for i in range(2)]
            for h in range(HPC):
                cc = h // 2
                ro = (h % 2) * 64
                for g4 in range(4):
                    ot_ps = psOT.tile([65, 512], fp32)
                    nkt = g4 * 4 + 4
                    for kt in range(nkt):
                        diag = (kt // 4 == g4)
                        off_in = 128 * (kt % 4) if diag else 0
                        span = 512 - off_in
                        st_ps = psS.tile([128, 512], fp32)
                        nc.tensor.matmul(
                            st_ps[:, off_in:],
                            kt_sb[cc][ro:ro + 64, kt * 128:(kt + 1) * 128],
                            qt_sb[cc][ro:ro + 64, g4 * 512 + off_in:(g4 + 1) * 512],
                            start=True,
                            stop=True,
                        )
                        pt = pt_pool.tile([128, 512], mm_dt)
                        nc.scalar.activation(
                            out=pt[:, off_in:],
                            in_=st_ps[:, off_in:],
                            func=mybir.ActivationFunctionType.Exp,
                            scale=0.125,
                        )
                        if diag:
                            if off_in:
                                nc.vector.memset(pt[:, :off_in], 0.0)
                            # keep iff f - p >= 0 (k index <= q index), else 0
                            nc.gpsimd.affine_select(
                                out=pt[:, off_in:],
                                in_=pt[:, off_in:],
                                compare_op=mybir.AluOpType.is_ge,
                                fill=0.0,
                                base=0,
                                channel_multiplier=-1,
                                pattern=[[1, span]],
                            )
                        nc.tensor.matmul(
                            ot_ps,
                            v_sb[:, kt, h, :],
                            pt,
                            start=(kt == 0),
                            stop=(kt == nkt - 1),
                        )
                    # divide by Z: reciprocal of row 64, broadcast via matmul
                    zr = zr_pool.tile([1, 512], fp32)
                    nc.vector.reciprocal(out=zr, in_=ot_ps[64:65, :])
                    zb_ps = psZ.tile([64, 512], fp32)
                    nc.tensor.matmul(zb_ps, ones_sb, zr, start=True, stop=True)
                    zb_sb = zbs_pool.tile([64, 512], fp32)
                    nc.vector.tensor_copy(out=zb_sb, in_=zb_ps)
                    nc.vector.tensor_mul(
                        out=ot_sb[cc][ro:ro + 64, g4 * 512:(g4 + 1) * 512],
                        in0=ot_ps[0:64, :],
                        in1=zb_sb,
                    )

            # ---- stage C: output projection ---------------------------------
            y_view = y.rearrange("(lt p) c -> p lt c", p=128)
            for qc in range(4):
                yt = y_pool.tile([128, 4, D], fp32)
                for li in range(4):
                    lt = qc * 4 + li
                    for dg in range(2):
                        ps = psA.tile([128, 512], fp32)
                        for cc in range(2):
                            nc.tensor.matmul(
                                ps,
                                ot_sb[cc][:, lt * 128:(lt + 1) * 128],
                                wo_sb[cc][:, dg * 512:(dg + 1) * 512],
                                start=(cc == 0),
                                stop=(cc == 1),
                            )
                        nc.vector.tensor_copy(
                            out=yt[:, li, dg * 512:(dg + 1) * 512],
                            in_=ps,
                        )
                nc.sync.dma_start(
                    out=y_view[:, qc * 4:(qc + 1) * 4, :],
                    in_=yt,
                )

    nc.compile()
    return nc


def _get_nc(mm_dt: str):
    if mm_dt not in _CACHE:
        _CACHE[mm_dt] = build_nc(mm_dt)
    return _CACHE[mm_dt]


def kernel(q, k, v, mask, Wq, bq, Wk, bk, Wv, bv, Wo, bo, _trace=False):
    nc = _get_nc(MM_DT)

    in_maps = []
    for c in range(NCORES):
        b = c // 4
        g = c % 4
        s = slice(g * CPC, (g + 1) * CPC)
        in_maps.append({
            "xq": np.ascontiguousarray(q[b].T).astype(NP_MM),
            "xk": np.ascontiguousarray(k[b].T).astype(NP_MM),
            "xv": np.ascontiguousarray(v[b].T).astype(NP_MM),
            "wq": np.ascontiguousarray(Wq[:, s]).astype(NP_MM),
            "wk": np.ascontiguousarray(Wk[:, s]).astype(NP_MM),
            "wv": np.ascontiguousarray(Wv[:, s]).astype(NP_MM),
            "wo": np.ascontiguousarray(Wo[s, :]).astype(NP_MM),
            "bq": np.ascontiguousarray(bq[s]).reshape(CPC, 1).astype(np.float32),
            "bk": np.ascontiguousarray(bk[s]).reshape(CPC, 1).astype(np.float32),
        })

    res = run_bass_kernel_spmd(nc, in_maps, list(range(NCORES)), trace=_trace)

    # host gather: out[b] = sum_g y_core(b,g) + (bo + bv @ Wo)
    const = (bo + bv.astype(np.float64) @ Wo.astype(np.float64)).astype(np.float64)
    out = np.zeros((B, L, D), np.float64)
    for c in range(NCORES):
        out[c // 4] += res.results[c]["y"].astype(np.float64)
    out += const[None, None, :]
    kernel.last_exec_time_ns = res.exec_time_ns
    return out.astype(np.float32)


# revision 22
# speedup vs baseline: 1.5497x; 1.5497x over previous
import sys

sys.path.insert(0, "/opt/trn_rl_repo")

import numpy as np
import ml_dtypes

import concourse.bass as bass
import concourse.bacc as bacc
import concourse.tile as tile
from concourse.bass_utils import run_bass_kernel_spmd
from concourse import mybir

B, L, D, H = 2, 2048, 1024, 16
DH = 64          # dim per head
HPC = 4          # heads per core
CPC = HPC * DH   # feature cols per core = 256
NCORES = 8

MM_DT = "bfloat16"
NP_MM = ml_dtypes.bfloat16 if MM_DT == "bfloat16" else np.float32

_CACHE = {}


def build_nc(mm_dt: str):
    nc = bacc.Bacc()
    mm_dt = mybir.dt(mm_dt)
    fp32 = mybir.dt.float32

    xq = nc.declare_dram_parameter("xq", (D, L), mm_dt, isOutput=False)   # q[b].T
    xk = nc.declare_dram_parameter("xk", (D, L), mm_dt, isOutput=False)   # k[b].T
    xv = nc.declare_dram_parameter("xv", (D, L), mm_dt, isOutput=False)   # v[b].T
    wq = nc.declare_dram_parameter("wq", (D, CPC), mm_dt, isOutput=False)
    wk = nc.declare_dram_parameter("wk", (D, CPC), mm_dt, isOutput=False)
    wv = nc.declare_dram_parameter("wv", (D, CPC), mm_dt, isOutput=False)
    wo = nc.declare_dram_parameter("wo", (CPC, D), mm_dt, isOutput=False)
    bq = nc.declare_dram_parameter("bq", (CPC, 1), fp32, isOutput=False)
    bk = nc.declare_dram_parameter("bk", (CPC, 1), fp32, isOutput=False)
    y = nc.declare_dram_parameter("y", (L, D), fp32, isOutput=True)       # partial out

    from contextlib import ExitStack

    with ExitStack() as es:
        tc = es.enter_context(tile.TileContext(nc))
        # NOTE: bufs are per named tag
        xt_pool = es.enter_context(tc.tile_pool(name="xt", bufs=1))     # 3 tags [128,8,2048]
        w_pool = es.enter_context(tc.tile_pool(name="w", bufs=1))       # 3 tags [128,8,256]
        wo_pool = es.enter_context(tc.tile_pool(name="wo", bufs=1))     # 2 tags [128,1024]
        bias_pool = es.enter_context(tc.tile_pool(name="bias", bufs=1))
        qt_pool = es.enter_context(tc.tile_pool(name="qt", bufs=1))     # 2 tags [128,2048]
        kt_pool = es.enter_context(tc.tile_pool(name="kt", bufs=1))
        vn_pool = es.enter_context(tc.tile_pool(name="vn", bufs=1))     # [128,16,4,65]
        pt_pool = es.enter_context(tc.tile_pool(name="pt", bufs=3))     # [128,512]
        zr_pool = es.enter_context(tc.tile_pool(name="zr", bufs=2))     # [1,512]
        zbs_pool = es.enter_context(tc.tile_pool(name="zbs", bufs=2))   # [64,512]
        ot_pool = es.enter_context(tc.tile_pool(name="ot", bufs=1))     # 2 tags [128,2048]
        y_pool = es.enter_context(tc.tile_pool(name="ysb", bufs=2))     # [128,4,1024]
        psA = es.enter_context(tc.tile_pool(name="psA", bufs=2, space="PSUM"))
        psS = es.enter_context(tc.tile_pool(name="psS", bufs=3, space="PSUM"))
        psOT = es.enter_context(tc.tile_pool(name="psOT", bufs=2, space="PSUM"))
        if True:
            # ---- load inputs (DMA queue order == consumption order) ---------
            wq_sb = w_pool.tile([128, 8, CPC], mm_dt, name="wq")
            nc.sync.dma_start(out=wq_sb, in_=wq.rearrange("(dc p) c -> p dc c", p=128))
            bq_sb = bias_pool.tile([128, 2], fp32, name="bq")
            nc.sync.dma_start(out=bq_sb, in_=bq.rearrange("(cc p) o -> p cc o", p=128))
            xq_sb = xt_pool.tile([128, 8, L], mm_dt, name="xq")
            xq_r = xq.rearrange("(dc p) c -> p dc c", p=128)
            for ch in range(4):
                nc.sync.dma_start(
                    out=xq_sb[:, 2 * ch:2 * ch + 2, :],
                    in_=xq_r[:, 2 * ch:2 * ch + 2, :],
                )

            wk_sb = w_pool.tile([128, 8, CPC], mm_dt, name="wk")
            nc.sync.dma_start(out=wk_sb, in_=wk.rearrange("(dc p) c -> p dc c", p=128))
            bk_sb = bias_pool.tile([128, 2], fp32, name="bk")
            nc.sync.dma_start(out=bk_sb, in_=bk.rearrange("(cc p) o -> p cc o", p=128))
            xk_sb = xt_pool.tile([128, 8, L], mm_dt, name="xk")
            xk_r = xk.rearrange("(dc p) c -> p dc c", p=128)
            for ch in range(4):
                nc.sync.dma_start(
                    out=xk_sb[:, 2 * ch:2 * ch + 2, :],
                    in_=xk_r[:, 2 * ch:2 * ch + 2, :],
                )

            wv_sb = w_pool.tile([128, 8, CPC], mm_dt, name="wv")
            nc.sync.dma_start(out=wv_sb, in_=wv.rearrange("(dc p) c -> p dc c", p=128))
            xv_sb = xt_pool.tile([128, 8, L], mm_dt, name="xv")
            xv_r = xv.rearrange("(dc p) c -> p dc c", p=128)
            for ch in range(4):
                nc.sync.dma_start(
                    out=xv_sb[:, 2 * ch:2 * ch + 2, :],
                    in_=xv_r[:, 2 * ch:2 * ch + 2, :],
                )

            wo_sb = []
            for cc in range(2):
                t = wo_pool.tile([128, D], mm_dt, name=f"wo{cc}")
                nc.sync.dma_start(out=t, in_=wo[cc * 128:(cc + 1) * 128, :])
                wo_sb.append(t)

            # ---- stage A: projections ---------------------------------------
            qt_sb = [qt_pool.tile([128, L], mm_dt, name=f"qt{i}") for i in range(2)]
            kt_sb = [kt_pool.tile([128, L], mm_dt, name=f"kt{i}") for i in range(2)]
            for dst, x_sb, w_sb, b_sb in (
                (qt_sb, xq_sb, wq_sb, bq_sb),
                (kt_sb, xk_sb, wk_sb, bk_sb),
            ):
                for cc in range(2):
                    for lg in range(4):
                        ps = psA.tile([128, 512], fp32)
                        for dc in range(8):
                            nc.tensor.matmul(
                                ps,
                                w_sb[:, dc, cc * 128:(cc + 1) * 128],
                                x_sb[:, dc, lg * 512:(lg + 1) * 512],
                                start=(dc == 0),
                                stop=(dc == 7),
                            )
                        nc.vector.tensor_scalar_add(
                            out=dst[cc][:, lg * 512:(lg + 1) * 512],
                            in0=ps,
                            scalar1=b_sb[:, cc:cc + 1],
                        )

            # V natural layout: [128(lt-part), 16 lt, 4 head, 65] (col 64 = ones)
            v_sb = vn_pool.tile([128, 16, 4, 65], mm_dt)
            nc.vector.memset(v_sb[:, :, :, 64:65], 1.0)
            for lt in range(16):
                ps = psA.tile([128, CPC], fp32)
                for dc in range(8):
                    nc.tensor.matmul(
                        ps,
                        xv_sb[:, dc, lt * 128:(lt + 1) * 128],
                        wv_sb[:, dc, :],
                        start=(dc == 0),
                        stop=(dc == 7),
                    )
                nc.vector.tensor_copy(
                    out=v_sb[:, lt, :, 0:64],
                    in_=ps.rearrange("p (h d) -> p h d", d=64),
                )

            # ---- stage B + C interleaved ------------------------------------
            ot_sb = [ot_pool.tile([128, L], mm_dt, name=f"ot{i}") for i in range(2)]
            y_view = y.rearrange("(lt p) c -> p lt c", p=128)

            def emit_C(g4):
                yt = y_pool.tile([128, 4, D], fp32)
                for li in range(4):
                    lt = g4 * 4 + li
                    for dg in range(2):
                        ps = psA.tile([128, 512], fp32)
                        for cc in range(2):
                            nc.tensor.matmul(
                                ps,
                                ot_sb[cc][:, lt * 128:(lt + 1) * 128],
                                wo_sb[cc][:, dg * 512:(dg + 1) * 512],
                                start=(cc == 0),
                                stop=(cc == 1),
                            )
                        if dg == 0:
                            nc.vector.tensor_copy(
                                out=yt[:, li, dg * 512:(dg + 1) * 512], in_=ps
                            )
                        else:
                            nc.scalar.activation(
                                out=yt[:, li, dg * 512:(dg + 1) * 512],
                                in_=ps,
                                func=mybir.ActivationFunctionType.Copy,
                            )
                nc.sync.dma_start(out=y_view[:, g4 * 4:(g4 + 1) * 4, :], in_=yt)

            DEPTH = 2
            for g4 in range(4):
                for h in range(HPC):
                    cc = h // 2
                    ro = (h % 2) * 64
                    nkt = g4 * 4 + 4
                    ot_ps = psOT.tile([65, 512], fp32)
                    pts = {}

                    def emit_S(kt):
                        diag = (kt // 4 == g4)
                        off = 128 * (kt % 4) if diag else 0
                        st = psS.tile([128, 512], fp32)
                        nc.tensor.matmul(
                            st[:, off:],
                            kt_sb[cc][ro:ro + 64, kt * 128:(kt + 1) * 128],
                            qt_sb[cc][ro:ro + 64, g4 * 512 + off:(g4 + 1) * 512],
                            start=True,
                            stop=True,
                        )
                        pt = pt_pool.tile([128, 512], mm_dt)
                        nc.scalar.activation(
                            out=pt,
                            in_=st,
                            func=mybir.ActivationFunctionType.Exp,
                            scale=0.125,
                        )
                        if diag:
                            # keep iff f - p - off >= 0 (covers both the causal
                            # mask and zero-filling cols [0:off) left stale by
                            # the partial-width S^T)
                            nc.gpsimd.affine_select(
                                out=pt,
                                in_=pt,
                                compare_op=mybir.AluOpType.is_ge,
                                fill=0.0,
                                base=-off,
                                channel_multiplier=-1,
                                pattern=[[1, 512]],
                            )
                        pts[kt] = pt

                    def emit_P(kt):
                        nc.tensor.matmul(
                            ot_ps,
                            v_sb[:, kt, h, :],
                            pts.pop(kt),
                            start=(kt == 0),
                            stop=(kt == nkt - 1),
                        )

                    for kt in range(nkt):
                        emit_S(kt)
                        if kt >= DEPTH:
                            emit_P(kt - DEPTH)
                    for kt in range(max(0, nkt - DEPTH), nkt):
                        emit_P(kt)

                    # divide by Z (row 64) -- off the PE entirely.
                    # NB: reciprocal_approx_fast reading PSUM directly is
                    # silently wrong; bounce the row through SBUF first.
                    zrow = zr_pool.tile([1, 512], fp32, name="zrow")
                    nc.vector.tensor_copy(out=zrow, in_=ot_ps[64:65, :])
                    zr = zr_pool.tile([1, 512], fp32, name="zr")
                    nc.vector.reciprocal_approx_fast(out=zr, in_=zrow)
                    zb = zbs_pool.tile([64, 512], fp32)
                    nc.gpsimd.partition_broadcast(out_ap=zb, in_ap=zr)
                    nc.vector.tensor_mul(
                        out=ot_sb[cc][ro:ro + 64, g4 * 512:(g4 + 1) * 512],
                        in0=ot_ps[0:64, :],
                        in1=zb,
                    )
                    if h == 0 and g4 > 0:
                        emit_C(g4 - 1)
            emit_C(3)

    nc.compile()
    return nc


def _get_nc(mm_dt: str):
    if mm_dt not in _CACHE:
        _CACHE[mm_dt] = build_nc(mm_dt)
    return _CACHE[mm_dt]


def kernel(q, k, v, mask, Wq, bq, Wk, bk, Wv, bv, Wo, bo, _trace=False):
    nc = _get_nc(MM_DT)

    in_maps = []
    for c in range(NCORES):
        b = c // 4
        g = c % 4
        s = slice(g * CPC, (g + 1) * CPC)
        in_maps.append({
            "xq": np.ascontiguousarray(q[b].T).astype(NP_MM),
            "xk": np.ascontiguousarray(k[b].T).astype(NP_MM),
            "xv": np.ascontiguousarray(v[b].T).astype(NP_MM),
            "wq": np.ascontiguousarray(Wq[:, s]).astype(NP_MM),
            "wk": np.ascontiguousarray(Wk[:, s]).astype(NP_MM),
            "wv": np.ascontiguousarray(Wv[:, s]).astype(NP_MM),
            "wo": np.ascontiguousarray(Wo[s, :]).astype(NP_MM),
            "bq": np.ascontiguousarray(bq[s]).reshape(CPC, 1).astype(np.float32),
            "bk": np.ascontiguousarray(bk[s]).reshape(CPC, 1).astype(np.float32),
        })

    res = run_bass_kernel_spmd(nc, in_maps, list(range(NCORES)), trace=_trace)

    # host gather: out[b] = sum_g y_core(b,g) + (bo + bv @ Wo)
    const = (bo + bv.astype(np.float64) @ Wo.astype(np.float64)).astype(np.float64)
    out = np.zeros((B, L, D), np.float64)
    for c in range(NCORES):
        out[c // 4] += res.results[c]["y"].astype(np.float64)
    out += const[None, None, :]
    kernel.last_exec_time_ns = res.exec_time_ns
    return out.astype(np.float32)


# revision 28
# speedup vs baseline: 1.6271x; 1.0499x over previous
import sys

sys.path.insert(0, "/opt/trn_rl_repo")

import numpy as np
import ml_dtypes

import concourse.bass as bass
import concourse.bacc as bacc
import concourse.tile as tile
from concourse.bass_utils import run_bass_kernel_spmd
from concourse import mybir

B, L, D, H = 2, 2048, 1024, 16
DH = 64          # dim per head
HPC = 4          # heads per core
CPC = HPC * DH   # feature cols per core = 256
NCORES = 8

MM_DT = "bfloat16"
NP_MM = ml_dtypes.bfloat16 if MM_DT == "bfloat16" else np.float32

_CACHE = {}


def build_nc(mm_dt: str):
    nc = bacc.Bacc()
    mm_dt = mybir.dt(mm_dt)
    fp32 = mybir.dt.float32

    xq = nc.declare_dram_parameter("xq", (D, L), mm_dt, isOutput=False)   # q[b].T
    xk = nc.declare_dram_parameter("xk", (D, L), mm_dt, isOutput=False)   # k[b].T
    xv = nc.declare_dram_parameter("xv", (D, L), mm_dt, isOutput=False)   # v[b].T
    wq = nc.declare_dram_parameter("wq", (D, CPC), mm_dt, isOutput=False)
    wk = nc.declare_dram_parameter("wk", (D, CPC), mm_dt, isOutput=False)
    wv = nc.declare_dram_parameter("wv", (D, CPC), mm_dt, isOutput=False)
    wo = nc.declare_dram_parameter("wo", (CPC, D), mm_dt, isOutput=False)
    bq = nc.declare_dram_parameter("bq", (CPC, 1), fp32, isOutput=False)
    bk = nc.declare_dram_parameter("bk", (CPC, 1), fp32, isOutput=False)
    y = nc.declare_dram_parameter("y", (L, D), fp32, isOutput=True)       # partial out

    from contextlib import ExitStack

    with ExitStack() as es:
        tc = es.enter_context(tile.TileContext(nc))
        # NOTE: bufs are per named tag
        xt_pool = es.enter_context(tc.tile_pool(name="xt", bufs=1))     # 3 tags [128,8,2048]
        w_pool = es.enter_context(tc.tile_pool(name="w", bufs=1))       # 3 tags [128,8,256]
        wo_pool = es.enter_context(tc.tile_pool(name="wo", bufs=1))     # 2 tags [128,1024]
        bias_pool = es.enter_context(tc.tile_pool(name="bias", bufs=1))
        qt_pool = es.enter_context(tc.tile_pool(name="qt", bufs=1))     # 2 tags [128,2048]
        kt_pool = es.enter_context(tc.tile_pool(name="kt", bufs=1))
        vn_pool = es.enter_context(tc.tile_pool(name="vn", bufs=1))     # [128,16,4,65]
        pt_pool = es.enter_context(tc.tile_pool(name="pt", bufs=6))     # [128,512]
        zr_pool = es.enter_context(tc.tile_pool(name="zr", bufs=3))     # [1,512]
        zbs_pool = es.enter_context(tc.tile_pool(name="zbs", bufs=3))   # [64,512]
        ot_pool = es.enter_context(tc.tile_pool(name="ot", bufs=1))     # 2 tags [128,2048]
        y_pool = es.enter_context(tc.tile_pool(name="ysb", bufs=2))     # [128,4,1024]
        psA = es.enter_context(tc.tile_pool(name="psA", bufs=2, space="PSUM"))
        psS = es.enter_context(tc.tile_pool(name="psS", bufs=4, space="PSUM"))
        psOT = es.enter_context(tc.tile_pool(name="psOT", bufs=2, space="PSUM"))
        if True:
            # ---- load inputs (DMA queue order == consumption order) ---------
            wq_sb = w_pool.tile([128, 8, CPC], mm_dt, name="wq")
            nc.sync.dma_start(out=wq_sb, in_=wq.rearrange("(dc p) c -> p dc c", p=128))
            bq_sb = bias_pool.tile([128, 2], fp32, name="bq")
            nc.sync.dma_start(out=bq_sb, in_=bq.rearrange("(cc p) o -> p cc o", p=128))
            xq_sb = xt_pool.tile([128, 8, L], mm_dt, name="xq")
            xq_r = xq.rearrange("(dc p) c -> p dc c", p=128)
            for ch in range(4):
                nc.sync.dma_start(
                    out=xq_sb[:, :, 512 * ch:512 * ch + 512],
                    in_=xq_r[:, :, 512 * ch:512 * ch + 512],
                )

            wk_sb = w_pool.tile([128, 8, CPC], mm_dt, name="wk")
            nc.sync.dma_start(out=wk_sb, in_=wk.rearrange("(dc p) c -> p dc c", p=128))
            bk_sb = bias_pool.tile([128, 2], fp32, name="bk")
            nc.sync.dma_start(out=bk_sb, in_=bk.rearrange("(cc p) o -> p cc o", p=128))
            xk_sb = xt_pool.tile([128, 8, L], mm_dt, name="xk")
            xk_r = xk.rearrange("(dc p) c -> p dc c", p=128)
            for ch in range(4):
                nc.sync.dma_start(
                    out=xk_sb[:, :, 512 * ch:512 * ch + 512],
                    in_=xk_r[:, :, 512 * ch:512 * ch + 512],
                )

            wv_sb = w_pool.tile([128, 8, CPC], mm_dt, name="wv")
            nc.sync.dma_start(out=wv_sb, in_=wv.rearrange("(dc p) c -> p dc c", p=128))
            xv_sb = xt_pool.tile([128, 8, L], mm_dt, name="xv")
            xv_r = xv.rearrange("(dc p) c -> p dc c", p=128)
            for ch in range(4):
                nc.sync.dma_start(
                    out=xv_sb[:, :, 512 * ch:512 * ch + 512],
                    in_=xv_r[:, :, 512 * ch:512 * ch + 512],
                )

            wo_sb = []
            for cc in range(2):
                t = wo_pool.tile([128, D], mm_dt, name=f"wo{cc}")
                nc.sync.dma_start(out=t, in_=wo[cc * 128:(cc + 1) * 128, :])
                wo_sb.append(t)

            # ---- stage A: projections ---------------------------------------
            qt_sb = [qt_pool.tile([128, L], mm_dt, name=f"qt{i}") for i in range(2)]
            kt_sb = [kt_pool.tile([128, L], mm_dt, name=f"kt{i}") for i in range(2)]
            for dst, x_sb, w_sb, b_sb in (
                (qt_sb, xq_sb, wq_sb, bq_sb),
                (kt_sb, xk_sb, wk_sb, bk_sb),
            ):
                for cc in range(2):
                    for lg in range(4):
                        ps = psA.tile([128, 512], fp32)
                        for dc in range(8):
                            nc.tensor.matmul(
                                ps,
                                w_sb[:, dc, cc * 128:(cc + 1) * 128],
                                x_sb[:, dc, lg * 512:(lg + 1) * 512],
                                start=(dc == 0),
                                stop=(dc == 7),
                            )
                        nc.vector.tensor_scalar_add(
                            out=dst[cc][:, lg * 512:(lg + 1) * 512],
                            in0=ps,
                            scalar1=b_sb[:, cc:cc + 1],
                        )

            # V natural layout: [128(lt-part), 16 lt, 4 head, 65] (col 64 = ones)
            v_sb = vn_pool.tile([128, 16, 4, 65], mm_dt)
            nc.vector.memset(v_sb[:, :, :, 64:65], 1.0)
            for lt in range(16):
                ps = psA.tile([128, CPC], fp32)
                for dc in range(8):
                    nc.tensor.matmul(
                        ps,
                        xv_sb[:, dc, lt * 128:(lt + 1) * 128],
                        wv_sb[:, dc, :],
                        start=(dc == 0),
                        stop=(dc == 7),
                    )
                nc.vector.tensor_copy(
                    out=v_sb[:, lt, :, 0:64],
                    in_=ps.rearrange("p (h d) -> p h d", d=64),
                )

            # ---- stage B + C interleaved ------------------------------------
            ot_sb = [ot_pool.tile([128, L], mm_dt, name=f"ot{i}") for i in range(2)]
            y_view = y.rearrange("(lt p) c -> p lt c", p=128)

            def emit_C(g4):
                yt = y_pool.tile([128, 4, D], fp32)
                for li in range(4):
                    lt = g4 * 4 + li
                    for dg in range(2):
                        ps = psA.tile([128, 512], fp32)
                        for cc in range(2):
                            nc.tensor.matmul(
                                ps,
                                ot_sb[cc][:, lt * 128:(lt + 1) * 128],
                                wo_sb[cc][:, dg * 512:(dg + 1) * 512],
                                start=(cc == 0),
                                stop=(cc == 1),
                            )
                        if dg == 0:
                            nc.vector.tensor_copy(
                                out=yt[:, li, dg * 512:(dg + 1) * 512], in_=ps
                            )
                        else:
                            nc.scalar.activation(
                                out=yt[:, li, dg * 512:(dg + 1) * 512],
                                in_=ps,
                                func=mybir.ActivationFunctionType.Copy,
                            )
                nc.sync.dma_start(out=y_view[:, g4 * 4:(g4 + 1) * 4, :], in_=yt)

            DEPTH = 2
            for g4 in range(4):
                for h in range(HPC):
                    cc = h // 2
                    ro = (h % 2) * 64
                    nkt = g4 * 4 + 4
                    ot_ps = psOT.tile([65, 512], fp32)
                    pts = {}

                    def emit_S(kt):
                        diag = (kt // 4 == g4)
                        off = 128 * (kt % 4) if diag else 0
                        st = psS.tile([128, 512], fp32)
                        nc.tensor.matmul(
                            st[:, off:],
                            kt_sb[cc][ro:ro + 64, kt * 128:(kt + 1) * 128],
                            qt_sb[cc][ro:ro + 64, g4 * 512 + off:(g4 + 1) * 512],
                            start=True,
                            stop=True,
                        )
                        pt = pt_pool.tile([128, 512], mm_dt)
                        nc.scalar.activation(
                            out=pt[:, off:],
                            in_=st[:, off:],
                            func=mybir.ActivationFunctionType.Exp,
                            scale=0.125,
                        )
                        if diag:
                            # keep iff f - p - off >= 0. Cols >= off+128 are
                            # all-keep (skip); cols < off are all-fill (zeroes
                            # the stale region the partial-width exp skipped).
                            w = off + 128
                            nc.gpsimd.affine_select(
                                out=pt[:, :w],
                                in_=pt[:, :w],
                                compare_op=mybir.AluOpType.is_ge,
                                fill=0.0,
                                base=-off,
                                channel_multiplier=-1,
                                pattern=[[1, w]],
                            )
                        pts[kt] = pt

                    def emit_P(kt):
                        nc.tensor.matmul(
                            ot_ps,
                            v_sb[:, kt, h, :],
                            pts.pop(kt),
                            start=(kt == 0),
                            stop=(kt == nkt - 1),
                        )

                    for kt in range(nkt):
                        emit_S(kt)
                        if kt >= DEPTH:
                            emit_P(kt - DEPTH)
                    for kt in range(max(0, nkt - DEPTH), nkt):
                        emit_P(kt)

                    # divide by Z (row 64) -- off the PE entirely.
                    # NB: reciprocal_approx_fast reading PSUM directly is
                    # silently wrong; bounce the row through SBUF first.
                    zrow = zr_pool.tile([1, 512], fp32, name="zrow")
                    nc.vector.tensor_copy(out=zrow, in_=ot_ps[64:65, :])
                    zr = zr_pool.tile([1, 512], fp32, name="zr")
                    nc.vector.reciprocal_approx_fast(out=zr, in_=zrow)
                    zb = zbs_pool.tile([64, 512], fp32)
                    nc.gpsimd.partition_broadcast(out_ap=zb, in_ap=zr)
                    nc.vector.tensor_mul(
                        out=ot_sb[cc][ro:ro + 64, g4 * 512:(g4 + 1) * 512],
                        in0=ot_ps[0:64, :],
                        in1=zb,
                    )
                    if h == 0 and g4 > 0:
                        emit_C(g4 - 1)
            emit_C(3)

    nc.compile()
    return nc


def _get_nc(mm_dt: str):
    if mm_dt not in _CACHE:
        _CACHE[mm_dt] = build_nc(mm_dt)
    return _CACHE[mm_dt]


def kernel(q, k, v, mask, Wq, bq, Wk, bk, Wv, bv, Wo, bo, _trace=False):
    nc = _get_nc(MM_DT)

    in_maps = []
    for c in range(NCORES):
        b = c // 4
        g = c % 4
        s = slice(g * CPC, (g + 1) * CPC)
        in_maps.append({
            "xq": np.ascontiguousarray(q[b].T).astype(NP_MM),
            "xk": np.ascontiguousarray(k[b].T).astype(NP_MM),
            "xv": np.ascontiguousarray(v[b].T).astype(NP_MM),
            "wq": np.ascontiguousarray(Wq[:, s]).astype(NP_MM),
            "wk": np.ascontiguousarray(Wk[:, s]).astype(NP_MM),
            "wv": np.ascontiguousarray(Wv[:, s]).astype(NP_MM),
            "wo": np.ascontiguousarray(Wo[s, :]).astype(NP_MM),
            "bq": np.ascontiguousarray(bq[s]).reshape(CPC, 1).astype(np.float32),
            "bk": np.ascontiguousarray(bk[s]).reshape(CPC, 1).astype(np.float32),
        })

    res = run_bass_kernel_spmd(nc, in_maps, list(range(NCORES)), trace=_trace)

    # host gather: out[b] = sum_g y_core(b,g) + (bo + bv @ Wo)
    const = (bo + bv.astype(np.float64) @ Wo.astype(np.float64)).astype(np.float64)
    out = np.zeros((B, L, D), np.float64)
    for c in range(NCORES):
        out[c // 4] += res.results[c]["y"].astype(np.float64)
    out += const[None, None, :]
    kernel.last_exec_time_ns = res.exec_time_ns
    return out.astype(np.float32)
